# revision 32
# baseline (speedup 1.0000x reference)
"""APPNP-over-GAT distributed Trainium2 kernel (8 NeuronCores), v3.

Sharding: tensor-parallel over (head, out_feat). Each core owns a 128-wide
slice of every head's 1024 out-features (3*128 = 384 local features).

v3 restructure vs v2 (224 us):
- xw projection stream split across BOTH HWDGE rings (sync + scalar)
  to lift effective DMA from ~200 GB/s toward the ~358 GB/s HBM cap.
- Degree/dinv chain moved to [128,4] ops + PE-transpose broadcast
  (kills the 4us single-partition [1,512] reciprocal and the 9.6us
  post-projection PE bubble).
- el/er AllReduce payload carries el in [3,512] row layout (PE-transposed
  pre-AR) and er in [128,12] partition layout: no 512-descriptor gather
  DMA and no fp32 broadcast matmuls after the AR. el broadcast via
  gpsimd.partition_broadcast.
- Full APPNP polynomial P^T = C^10 + 0.1*sum(C^i) (C = 0.9*Ahat^T) built
  in 9 [512x512] matmul rounds DURING the AllReduce window; the k=10
  chain collapses to ONE post-attention apply round.
- Exp/Identity activation tables preloaded via dummy ops right after the
  projection (1.5us ACT_TABLE_LOAD off the critical path).
- fc cross-partition sum via gpsimd.partition_all_reduce (no fp32 matmul).
- Warm-up collective removed (entry barrier already warms the CC stream).
"""

import os
import sys

sys.path.insert(0, "/opt/trn_rl_repo")

import numpy as np

N = 500
NP = 512  # padded nodes
F = 8192
H = 3
O = 1024
OL = 128  # out-features per head per core
SH = H * OL  # 384 local features
KF = F // 128  # 64 k-tiles
G = 4  # k-tiles per DMA group
NG = KF // G  # 16 groups
XB = G * NP  # 2048: x block cols per group tile
WB = G * SH  # 1536: w block cols per group tile
NC = 8
K_STEPS = 10
ALPHA = 0.1
NEG_SLOPE = 0.2

LAST_EXEC_NS = None
LAST_RESULT = None


def build(stage=99):
    import concourse.bacc as bacc
    import concourse.mybir as mybir
    import concourse.tile as tile
    from concourse import bass_isa
    from concourse.masks import make_identity

    f32 = mybir.dt.float32
    bf16 = mybir.dt.bfloat16
    Alu = mybir.AluOpType
    Act = mybir.ActivationFunctionType
    AX = mybir.AxisListType.X

    nc = bacc.Bacc("TRN2", target_bir_lowering=False, debug=False, num_devices=NC)

    xw = nc.declare_dram_parameter("xw", [NG, 128, XB + WB], bf16, isOutput=False)
    aftp = nc.declare_dram_parameter("aftp", [128, 4 * NP], bf16, isOutput=False)
    afp = nc.declare_dram_parameter("afp", [128, 4 * NP], bf16, isOutput=False)
    attn = nc.declare_dram_parameter("attn", [128, 2 * SH], bf16, isOutput=False)
    fcwp = nc.declare_dram_parameter("fcwp", [128, 4 * 2 * SH], bf16, isOutput=False)
    fcb = nc.declare_dram_parameter("fcb", [1, 16], f32, isOutput=False)
    out_ext = nc.declare_dram_parameter("out", [1, 16], f32, isOutput=True)

    rg = [list(range(NC))]

    for _single_pass in range(1):
        with tile.TileContext(nc) as tc:
            with (
                tc.tile_pool(name="consts", bufs=1) as consts,
                tc.tile_pool(name="persist", bufs=1) as persist,
                tc.tile_pool(name="stream", bufs=3) as stream,
                tc.tile_pool(name="dram", bufs=1, space="DRAM") as dram,
            ):
                # ---- warm-up collective: absorbs the CC engine's cold-start
                # latency and cross-core launch skew, so the real AllReduce
                # later hits a warm CC pipeline. Triggered data-independently
                # at kernel entry.
                warm_in = dram.tile([1, 64], f32, name="warm_in", tag="warm_in")
                warm_out = dram.tile([1, 64], f32, name="warm_out", tag="warm_out")
                nc.gpsimd.collective_compute(
                    "AllReduce", Alu.add, ins=[warm_in.opt()], outs=[warm_out.opt()],
                    replica_groups=rg,
                )

                ones_col_b = consts.tile([128, 1], bf16, name="ones_col_b", tag="ocb")
                nc.gpsimd.memset(ones_col_b[:, :], 1.0)
                ones_row_b = consts.tile([1, 128], bf16, name="ones_row_b", tag="orb")
                nc.gpsimd.memset(ones_row_b[:, :], 1.0)
                ident_b = consts.tile([128, 128], bf16, name="ident_b", tag="identb")
                make_identity(nc, ident_b[:, :])
                ident_f = consts.tile([128, 128], f32, name="ident_f", tag="identf")
                make_identity(nc, ident_f[:, :])

                # ---- prologue loads (aft/af at the head of the two HWDGE
                # rings, ahead of the xw stream; small so they barely delay g0/g1)
                aft_sb = persist.tile([128, 4 * NP], bf16, name="aft_sb", tag="aft_sb")
                nc.sync.dma_start(aft_sb[:, :], aftp[:, :])
                af_sb = persist.tile([128, 4 * NP], bf16, name="af_sb", tag="af_sb")
                nc.scalar.dma_start(af_sb[:, :], afp[:, :])
                attn_sb = consts.tile([128, 2 * SH], bf16, name="attn_sb", tag="attn_sb")
                nc.gpsimd.dma_start(attn_sb[:, :], attn[:, :])
                fcb_sb = consts.tile([1, 16], f32, name="fcb_sb", tag="fcb_sb")
                nc.gpsimd.dma_start(fcb_sb[:, :], fcb[:, :])
                aft_t = [aft_sb[:, k * NP : (k + 1) * NP] for k in range(4)]
                af_t = [af_sb[:, k * NP : (k + 1) * NP] for k in range(4)]
                fcw_sb = persist.tile([128, 8 * SH], bf16, name="fcw_sb", tag="fcw_sb")
                fcw_t = [fcw_sb[:, m * 2 * SH : (m + 1) * 2 * SH] for m in range(4)]

                ppA = tc.tile_pool(name="psumA", bufs=1, space="PSUM")
                pp = ppA.__enter__()

                # ---- degrees: dc[p, m] = row-sum of A over node block m (PE)
                dc_psum = pp.tile([128, 4], f32, name="dc_psum", tag="dc")
                for m in range(4):
                    for k in range(4):
                        nc.tensor.matmul(
                            dc_psum[:, m : m + 1],
                            aft_t[k][:, m * 128 : (m + 1) * 128],
                            ones_col_b[:, :],
                            start=(k == 0), stop=(k == 3),
                        )
                dinvc = persist.tile([128, 4], f32, name="dinvc", tag="dinvc")
                dinvb09 = persist.tile([128, NP], f32, name="dinvb09", tag="dinvb09")

                def emit_dinv_a():
                    # dinv = rsqrt(deg+1). Emitted mid-loop so these waits
                    # never block the xw DMA issues.
                    sdc = stream.tile([128, 4], f32, name="sdc", tag="sdc")
                    nc.scalar.activation(sdc[:, :], dc_psum[:, :], Act.Sqrt, bias=1.0)
                    nc.vector.reciprocal(dinvc[:, :], sdc[:, :])

                def emit_dinv_b():
                    # broadcast dinv along the free dim, all on-chip: four
                    # single-column PE transposes build a [1,512] psum row
                    # (node order k*128+p), then gpsimd partition-broadcast.
                    dinvT = pp.tile([1, NP], f32, name="dinvT", tag="dinvT")
                    for k in range(4):
                        nc.tensor.transpose(
                            dinvT[:, k * 128 : (k + 1) * 128],
                            dinvc[:, k : k + 1], ident_f[:, :],
                        )
                    dinv_row = consts.tile([1, NP], f32, name="dinv_row", tag="dinv_row")
                    nc.scalar.copy(dinv_row[:, :], dinvT[:, :])
                    dvb = stream.tile([128, NP], f32, name="dvb", tag="dvb")
                    nc.gpsimd.partition_broadcast(dvb[:, :], dinv_row[:, :])
                    nc.scalar.mul(dinvb09[:, :], dvb[:, :], 1.0 - ALPHA)

                # ---- B twins (gpsimd, off the projection's DVE/scalar path)
                # bt = (0.9*Ahat)^T tiles (C), bu = 0.9*Ahat tiles (B)
                bt_sb = persist.tile([128, 4 * NP], bf16, name="bt_sb", tag="bt_sb")
                bu_sb = persist.tile([128, 4 * NP], bf16, name="bu_sb", tag="bu_sb")
                t2_sb = persist.tile([128, 4 * NP], bf16, name="t2_sb", tag="t2_sb")
                bt_t = [bt_sb[:, k * NP : (k + 1) * NP] for k in range(4)]
                bu_t = [bu_sb[:, k * NP : (k + 1) * NP] for k in range(4)]
                t2_t = [t2_sb[:, k * NP : (k + 1) * NP] for k in range(4)]
                d2 = persist.tile([128, 4], f32, name="d2", tag="d2")

                def emit_d2():
                    # self-loop diagonal correction 0.9*dinv^2 (A_sl = A + I)
                    nc.vector.tensor_mul(d2[:, :], dinvc[:, :], dinvc[:, :])
                    nc.vector.tensor_scalar_mul(d2[:, :], d2[:, :], 1.0 - ALPHA)

                def emit_b_prep(k):
                    # B twins from RAW A tiles (no in-place A+I mutation: that
                    # created a WAR serialization against every A reader).
                    # Emitted one k-tile per projection group to avoid a
                    # vector-engine burst that starves the wks chain.
                    dg = slice(k * NP + k * 128, k * NP + (k + 1) * 128)
                    nc.vector.scalar_tensor_tensor(
                        bt_t[k], aft_t[k], dinvc[:, k : k + 1], dinvb09[:, :],
                        op0=Alu.mult, op1=Alu.mult,
                    )
                    nc.vector.scalar_tensor_tensor(
                        bt_sb[:, dg], ident_b[:, :], d2[:, k : k + 1], bt_sb[:, dg],
                        op0=Alu.mult, op1=Alu.add,
                    )
                    nc.vector.scalar_tensor_tensor(
                        bu_t[k], af_t[k], dinvc[:, k : k + 1], dinvb09[:, :],
                        op0=Alu.mult, op1=Alu.mult,
                    )
                    nc.vector.scalar_tensor_tensor(
                        bu_sb[:, dg], ident_b[:, :], d2[:, k : k + 1], bu_sb[:, dg],
                        op0=Alu.mult, op1=Alu.add,
                    )

                def emit_t2(k):
                    # T2 = 0.1*(I + C)
                    dg = slice(k * NP + k * 128, k * NP + (k + 1) * 128)
                    nc.vector.tensor_scalar_mul(t2_t[k], bt_t[k], ALPHA)
                    nc.vector.scalar_tensor_tensor(
                        t2_sb[:, dg], ident_b[:, :], ALPHA, t2_sb[:, dg],
                        op0=Alu.mult, op1=Alu.add,
                    )

                # ---- fused projection stream: h = (x / l1colsum(x)) @ W
                # group tile: [128, x-tiles in order j0,j2,j1,j3 | w j0..j3]
                XOFF = [0, 2, 1, 3]
                rings = [nc.sync, nc.scalar]
                hp_psum = [pp.tile([128, SH], f32, name=f"hp{m}", tag=f"hp{m}") for m in range(4)]
                # pre-issue a 6-group DMA window so neither HWDGE ring idles
                # behind compute-queue waits
                PRE = 6
                xwts = [
                    stream.tile([128, XB + WB], bf16, name="xwt", tag="xwt", bufs=8)
                    for _ in range(NG)
                ]
                for g in range(PRE):
                    rings[g % 2].dma_start(xwts[g][:, :], xw[g, :, :])
                for g in range(NG):
                    # stagger each group at its realistic DMA-arrival time in
                    # the scheduler's virtual clock, so the static schedule
                    # doesn't hoist later groups' ops ahead of this group's
                    # wks chain (observed: that ordering stalls the PE)
                    grp_ctx = tc.tile_wait_until(0.012 + 0.0034 * g)
                    grp_ctx.__enter__()
                    if g + PRE < NG:
                        rings[(g + PRE) % 2].dma_start(xwts[g + PRE][:, :], xw[g + PRE, :, :])
                    if g == 1:
                        emit_dinv_a()
                    elif g == 2:
                        emit_dinv_b()
                    elif g == 3:
                        emit_d2()
                    elif 4 <= g <= 7:
                        emit_b_prep(g - 4)
                    elif 8 <= g <= 11:
                        emit_t2(g - 8)
                    xwt = xwts[g]
                    s_g = stream.tile([128, 4], f32, name="sg", tag="sg", bufs=4)
                    nc.vector.tensor_reduce(
                        s_g[:, 0:2],
                        xwt[:, 0 : 2 * NP].rearrange("p (j n) -> p j n", j=2),
                        axis=AX, op=Alu.add, apply_absolute_value=True,
                    )
                    for slot in (2, 3):
                        absj = stream.tile([128, NP], bf16, name="absj", tag="absj", bufs=4)
                        nc.scalar.activation(
                            absj[:, :], xwt[:, slot * NP : (slot + 1) * NP],
                            Act.Abs, accum_out=s_g[:, slot : slot + 1],
                        )
                    rs_g = stream.tile([128, 4], f32, name="rsg", tag="rsg", bufs=4)
                    nc.vector.reciprocal(rs_g[:, 0:2], s_g[:, 0:2])
                    nc.vector.reciprocal(rs_g[:, 2:4], s_g[:, 2:4])
                    for j in range(G):
                        k = g * G + j
                        sc = XOFF[j]
                        wk = xwt[:, XB + j * SH : XB + (j + 1) * SH]
                        wks = stream.tile([128, SH], bf16, name="wks", tag="wks", bufs=6)
                        if j == 3:
                            nc.scalar.mul(wks[:, :], wk, rs_g[:, sc : sc + 1])
                        else:
                            nc.vector.tensor_scalar_mul(wks[:, :], wk, rs_g[:, sc : sc + 1])
                        for m in range(4):
                            nc.tensor.matmul(
                                hp_psum[m][:, :],
                                xwt[:, XOFF[j] * NP + m * 128 : XOFF[j] * NP + (m + 1) * 128],
                                wks[:, :],
                                start=(k == 0),
                                stop=(k == KF - 1),
                            )
                    grp_ctx.__exit__(None, None, None)
                # late load: SWDGE trickle path, so neither HWDGE ring carries
                # it ahead of the xw stream (the tile scheduler hoists
                # dependency-free DMAs)
                nc.gpsimd.dma_start(fcw_sb[:, :], fcwp[:, :])
                # preload Identity/Exp activation tables off the critical path
                dummy = stream.tile([1, 1], f32, name="dummy", tag="dummy")
                nc.scalar.activation(dummy[:, :], ones_row_b[0:1, 0:1], Act.Identity)
                nc.scalar.activation(dummy[:, :], ones_row_b[0:1, 0:1], Act.Exp)

                # ---- el/er partial dots -> AR payload
                # payload [1, 3072] f32: [0:1536) el rows [3,512]; [1536:3072) er [128,12]
                ar_in = dram.tile([1, 6 * NP], f32, name="ar_in", tag="ar_in")
                ar_out = dram.tile([1, 6 * NP], f32, name="ar_out", tag="ar_out")
                el_pack = persist.tile([3, NP], f32, name="el_pack", tag="el_pack")
                er_pack = persist.tile([128, 12], f32, name="er_pack", tag="er_pack")
                for m in range(4):
                    prod = stream.tile([128, 2 * SH], bf16, name="elprod", tag="elprod")
                    nc.vector.tensor_mul(prod[:, 0:SH], hp_psum[m][:, :], attn_sb[:, 0:SH])
                    nc.vector.tensor_mul(prod[:, SH : 2 * SH], hp_psum[m][:, :], attn_sb[:, SH : 2 * SH])
                    eler_m = stream.tile([128, 6], f32, name="eler_m", tag="eler_m", bufs=4)
                    nc.vector.tensor_reduce(
                        eler_m[:, :],
                        prod.rearrange("p (s h o) -> p (s h) o", s=2, h=H),
                        axis=AX, op=Alu.add,
                    )
                    nc.scalar.copy(er_pack[:, m * 3 : (m + 1) * 3], eler_m[:, 3:6])
                    elt_psum = pp.tile([3, 128], f32, name="elt", tag="elt")
                    nc.tensor.transpose(elt_psum[:, :], eler_m[:, 0:3], ident_f[:, :])
                    nc.scalar.copy(el_pack[:, m * 128 : (m + 1) * 128], elt_psum[:, :])
                nc.sync.dma_start(
                    ar_in[0:1, 0 : 3 * NP].rearrange("p (r c) -> (p r) c", c=NP),
                    el_pack[:, :],
                )
                nc.sync.dma_start(
                    ar_in[0:1, 3 * NP : 6 * NP].rearrange("p (j c) -> (p j) c", c=12),
                    er_pack[:, :],
                )
                nc.gpsimd.collective_compute(
                    "AllReduce", Alu.add, ins=[ar_in.opt()], outs=[ar_out.opt()],
                    replica_groups=rg,
                )

                # h tiles with a ones column appended per head: [128, 3, 129]
                # (copied out during the AllReduce window)
                hp_sb = [
                    persist.tile([128, H, 129], bf16, name=f"hpsb{m}", tag=f"hpsb{m}")
                    for m in range(4)
                ]
                for m in range(4):
                    nc.gpsimd.memset(hp_sb[m][:, :, :], 1.0)
                    for h in range(H):
                        nc.scalar.copy(
                            hp_sb[m][:, h, 0:128],
                            hp_psum[m][:, h * OL : (h + 1) * OL],
                        )

                if stage <= 1:
                    resm = stream.tile([1, 16], f32, name="resm", tag="resm")
                    nc.gpsimd.memset(resm[:, :], 0.0)
                    nc.vector.tensor_copy(resm[:, 0:16], hp_sb[0][0:1, 0, 0:16])
                    nc.sync.dma_start(out_ext[:, :], resm[:, :])
                    ppA.__exit__(None, None, None)
                    break

                ppA.__exit__(None, None, None)
                ppB = tc.tile_pool(name="psumB", bufs=1, space="PSUM")
                pp = ppB.__enter__()

                # ---- P^T ladder during the AllReduce window.
                # C = B^T = (0.9 Ahat)^T; all operands commute (polynomials in C).
                # P^T = C8*(C2 + T2) + T8,  T2k = 0.1*sum_{i<2k} C^i
                def newmat(nm):
                    t = persist.tile([128, 4 * NP], bf16, name=nm, tag=nm)
                    return t, [t[:, k * NP : (k + 1) * NP] for k in range(4)]

                c2_sb, c2_t = newmat("c2")
                u2_sb, u2_t = newmat("u2")
                c4_sb, c4_t = newmat("c4")
                u4_sb, u4_t = newmat("u4")
                c8_sb, c8_t = newmat("c8")
                u8_sb, u8_t = newmat("u8")
                t4_sb, t4_t = newmat("t4")
                t8_sb, t8_t = newmat("t8")
                q_sb, q_t = newmat("q")
                pt_sb, pt_t = newmat("pt")

                def pround(dst, lhs, rhs, addend=None):
                    for m in range(4):
                        ps = pp.tile([128, NP], f32, name=f"pb{m}", tag=f"pb{m}")
                        for k in range(4):
                            nc.tensor.matmul(
                                ps[:, :],
                                lhs[k][:, m * 128 : (m + 1) * 128],
                                rhs[k][:, :],
                                start=(k == 0), stop=(k == 3),
                            )
                        if addend is not None:
                            nc.vector.scalar_tensor_tensor(
                                dst[m], ps[:, :], 1.0, addend[m],
                                op0=Alu.mult, op1=Alu.add,
                            )
                        elif m % 2 == 0:
                            nc.scalar.copy(dst[m], ps[:, :])
                        else:
                            nc.vector.tensor_copy(dst[m], ps[:, :])

                pround(c2_t, bu_t, bt_t)          # C^2
                pround(u2_t, bt_t, bu_t)          # B^2
                pround(c4_t, u2_t, c2_t)          # C^4
                pround(u4_t, c2_t, u2_t)          # B^4
                pround(t4_t, u2_t, t2_t, t2_t)    # T4 = C2*T2 + T2
                pround(c8_t, u4_t, c4_t)          # C^8
                pround(u8_t, c4_t, u4_t)          # B^8
                pround(t8_t, u4_t, t4_t, t4_t)    # T8 = C4*T4 + T4
                for k in range(4):
                    nc.gpsimd.tensor_add(q_t[k], c2_t[k], t2_t[k])  # Q = C2 + T2

                # ---- AllReduce readback: el rows + er partition layout
                el_row = [
                    persist.tile([1, NP], f32, name=f"el_row{h}", tag=f"el_row{h}")
                    for h in range(H)
                ]
                for h in range(H):
                    nc.sync.dma_start(el_row[h][:, :], ar_out[0:1, h * NP : (h + 1) * NP])
                er_all = persist.tile([128, 12], f32, name="er_all", tag="er_all")
                nc.sync.dma_start(
                    er_all[:, :],
                    ar_out[0:1, 3 * NP : 6 * NP].rearrange("p (j c) -> (p j) c", c=12),
                )
                elb = [
                    persist.tile([128, NP], f32, name=f"elb{h}", tag=f"elb{h}")
                    for h in range(H)
                ]
                for h in range(H):
                    nc.gpsimd.partition_broadcast(elb[h][:, :], el_row[h][:, :])

                if stage <= 2:
                    resm = stream.tile([1, 16], f32, name="resm", tag="resm")
                    nc.gpsimd.memset(resm[:, :], 0.0)
                    nc.vector.tensor_copy(resm[:, 0:6], er_all[0:1, 0:6])
                    nc.vector.tensor_copy(resm[:, 6:12], elb[0][0:1, 0:6])
                    nc.sync.dma_start(out_ext[:, :], resm[:, :])
                    ppB.__exit__(None, None, None)
                    break

                # ---- attention numerators, [src j, dst i] layout, bf16
                num_t = {}
                for h in range(H):
                    for k in range(4):
                        z_t = stream.tile([128, NP], bf16, name="zt", tag="zt", bufs=4)
                        if h < 2:
                            nc.scalar.activation(
                                z_t[:, :], elb[h][:, :], Act.Identity,
                                bias=er_all[:, k * 3 + h : k * 3 + h + 1],
                            )
                        else:
                            nc.vector.tensor_scalar_add(
                                z_t[:, :], elb[h][:, :], er_all[:, k * 3 + h : k * 3 + h + 1]
                            )
                        lr_t = stream.tile([128, NP], bf16, name="lrt", tag="lrt", bufs=4)
                        nc.vector.scalar_tensor_tensor(
                            lr_t[:, :], z_t[:, :], NEG_SLOPE, z_t[:, :],
                            op0=Alu.mult, op1=Alu.max,
                        )
                        ex_t = stream.tile([128, NP], bf16, name="ext", tag="ext", bufs=4)
                        nc.scalar.activation(ex_t[:, :], lr_t[:, :], Act.Exp)
                        numb = persist.tile([128, NP], bf16, name=f"num{h}_{k}", tag=f"num{h}_{k}")
                        nc.gpsimd.tensor_mul(numb[:, :], ex_t[:, :], aft_t[k])
                        num_t[(h, k)] = numb

                if stage <= 4:
                    resm = stream.tile([1, 16], f32, name="resm", tag="resm")
                    nc.gpsimd.memset(resm[:, :], 0.0)
                    nc.vector.tensor_copy(resm[:, 0:16], num_t[(2, 3)][0:1, 0:16])
                    nc.sync.dma_start(out_ext[:, :], resm[:, :])
                    ppB.__exit__(None, None, None)
                    break

                # ---- h0 = att @ h with fused denominators (ones column per head)
                h0_psum = [
                    pp.tile([128, H, 129], f32, name=f"h0p{m}", tag=f"h0p{m}")
                    for m in range(4)
                ]
                for m in range(4):
                    for h in range(H):
                        for k in range(4):
                            nc.tensor.matmul(
                                h0_psum[m][:, h, :],
                                num_t[(h, k)][:, m * 128 : (m + 1) * 128],
                                hp_sb[k][:, h, :],
                                start=(k == 0),
                                stop=(k == 3),
                            )
                h0_sb = [persist.tile([128, SH], bf16, name=f"h0sb{m}", tag=f"h0sb{m}") for m in range(4)]
                for m in range(4):
                    rdt = stream.tile([128, H], f32, name="rdt", tag="rdt", bufs=2)
                    nc.vector.tensor_scalar_add(rdt[:, :], h0_psum[m][:, :, 128], 1e-30)
                    rd = stream.tile([128, H], f32, name="rd", tag="rd", bufs=2)
                    nc.vector.reciprocal(rd[:, :], rdt[:, :])
                    for h in range(H):
                        if h % 2 == 0:
                            nc.scalar.mul(
                                h0_sb[m][:, h * OL : (h + 1) * OL],
                                h0_psum[m][:, h, 0:128],
                                rd[:, h : h + 1],
                            )
                        else:
                            nc.vector.tensor_scalar_mul(
                                h0_sb[m][:, h * OL : (h + 1) * OL],
                                h0_psum[m][:, h, 0:128],
                                rd[:, h : h + 1],
                            )

                if stage <= 5:
                    resm = stream.tile([1, 16], f32, name="resm", tag="resm")
                    nc.gpsimd.memset(resm[:, :], 0.0)
                    nc.vector.tensor_copy(resm[:, 0:16], h0_sb[0][0:1, 0:16])
                    nc.sync.dma_start(out_ext[:, :], resm[:, :])
                    ppB.__exit__(None, None, None)
                    break

                # final ladder round: P^T = C8*Q + T8 (queued after h0 matmuls)
                pround(pt_t, u8_t, q_t, t8_t)

                # ---- APPNP: hk = P @ h0 in ONE matmul round
                hc = [persist.tile([128, SH], bf16, name=f"hX{m}", tag=f"hX{m}") for m in range(4)]
                for m in range(4):
                    ps = pp.tile([128, NP], f32, name=f"pb{m}", tag=f"pb{m}")
                    for k in range(4):
                        nc.tensor.matmul(
                            ps[:, 0:SH],
                            pt_t[k][:, m * 128 : (m + 1) * 128],
                            h0_sb[k][:, :],
                            start=(k == 0), stop=(k == 3),
                        )
                    if m % 2 == 0:
                        nc.scalar.copy(hc[m][:, :], ps[:, 0:SH])
                    else:
                        nc.vector.tensor_copy(hc[m][:, :], ps[:, 0:SH])

                if stage <= 6:
                    resm = stream.tile([1, 16], f32, name="resm", tag="resm")
                    nc.gpsimd.memset(resm[:, :], 0.0)
                    nc.vector.tensor_copy(resm[:, 0:16], hc[0][0:1, 0:16])
                    nc.sync.dma_start(out_ext[:, :], resm[:, :])
                    ppB.__exit__(None, None, None)
                    break

                # ---- fc: partial dots + cross-partition sum + ReduceScatter
                parts = stream.tile([128, 8], f32, name="parts", tag="parts")
                for m in range(4):
                    for c in range(2):
                        junk = stream.tile([128, SH], bf16, name="fcjunk", tag="fcjunk", bufs=4)
                        nc.vector.tensor_mul(
                            junk[:, :], hc[m][:, :],
                            fcw_t[m][:, c * SH : (c + 1) * SH],
                        )
                        nc.vector.tensor_reduce(
                            parts[:, c * 4 + m : c * 4 + m + 1], junk[:, :],
                            axis=AX, op=Alu.add,
                        )
                # cross-partition sum of the 8 partials on the PE (a bf16
                # ones-matmul; avoids the gpsimd custom-op library swap)
                parts_b = stream.tile([128, 8], bf16, name="parts_b", tag="parts_b")
                nc.vector.tensor_copy(parts_b[:, :], parts[:, :])
                ppB.__exit__(None, None, None)
                ppC = tc.tile_pool(name="psumC", bufs=1, space="PSUM")
                ppc = ppC.__enter__()
                fin_psum = ppc.tile([1, 16], f32, name="fin", tag="fin")
                nc.tensor.matmul(
                    fin_psum[:, 0:8], ones_col_b[:, :], parts_b[:, :],
                    start=True, stop=True,
                )
                res16 = stream.tile([1, 16], f32, name="res16", tag="res16")
                nc.gpsimd.memset(res16[:, :], 0.0)
                nc.vector.tensor_copy(res16[:, 0:8], fin_psum[:, 0:8])
                ppC.__exit__(None, None, None)
                if stage <= 7:
                    nc.sync.dma_start(out_ext[:, :], res16[0:1, 0:16])
                    break
                # AllGather (lower latency floor than ReduceScatter) + local
                # sum of the 8 cores' partials; only core 0's out is read.
                fc_in = dram.tile([1, 16], f32, name="fc_in", tag="fc_in")
                fc_out = dram.tile([1, 128], f32, name="fc_out", tag="fc_out")
                nc.sync.dma_start(fc_in[:, :], res16[:, :])
                nc.gpsimd.collective_compute(
                    "AllGather", Alu.bypass, ins=[fc_in.opt()], outs=[fc_out.opt()],
                    replica_groups=rg,
                )
                # readback transposed: [1, value v, core c] so the core axis
                # is innermost for the reduce
                gath = stream.tile([1, 8, 8], f32, name="gath", tag="gath")
                nc.sync.dma_start(
                    gath[:, :, :],
                    fc_out[0:1, :].rearrange("p (c v) -> p v c", v=16)[:, 0:8, :],
                )
                res_f8 = stream.tile([1, 8], f32, name="resf8", tag="resf8")
                nc.vector.tensor_reduce(
                    res_f8[:, :], gath[:, :, :], axis=AX, op=Alu.add,
                )
                res_f = stream.tile([1, 16], f32, name="resf", tag="resf")
                nc.gpsimd.memset(res_f[:, :], 0.0)
                nc.vector.tensor_reduce(
                    res_f[:, 0:2], res_f8[0:1, :].rearrange("p (c m) -> p c m", c=2),
                    axis=AX, op=Alu.add,
                )
                nc.vector.tensor_add(res_f[:, :], res_f[:, :], fcb_sb[:, :])
                nc.sync.dma_start(out_ext[:, :], res_f[:, :])

    nc.finalize()
    return nc


def prepare_in_maps(A, x, W, attn_l, attn_r, fc_w, fc_b):
    import ml_dtypes

    bf16 = ml_dtypes.bfloat16
    A = np.asarray(A)
    x = np.asarray(x, dtype=np.float32)
    W = np.asarray(W, dtype=np.float32)
    attn_l = np.asarray(attn_l, dtype=np.float32)
    attn_r = np.asarray(attn_r, dtype=np.float32)
    fc_w = np.asarray(fc_w, dtype=np.float32)
    fc_b = np.asarray(fc_b, dtype=np.float32)

    xT = np.zeros((F, NP), dtype=bf16)
    xT[:, :N] = x.T.astype(bf16)
    # k-tile slot order [j0, j2, j1, j3] within each group (see XOFF)
    xg = np.ascontiguousarray(
        xT.reshape(NG, G, 128, NP)[:, [0, 2, 1, 3]]
        .transpose(0, 2, 1, 3)
        .reshape(NG, 128, XB)
    )
    aft = np.zeros((NP, NP), dtype=bf16)
    aft[:N, :N] = A.T.astype(bf16)
    aftp = np.ascontiguousarray(
        aft.reshape(4, 128, NP).transpose(1, 0, 2).reshape(128, 4 * NP)
    )
    af = np.zeros((NP, NP), dtype=bf16)
    af[:N, :N] = A.astype(bf16)
    afp = np.ascontiguousarray(
        af.reshape(4, 128, NP).transpose(1, 0, 2).reshape(128, 4 * NP)
    )
    fcb = np.zeros((1, 16), dtype=np.float32)
    fcb[0, :2] = fc_b
    fcv = fc_w.reshape(2, N, H, O)

    in_maps = []
    for c in range(NC):
        sl = slice(c * OL, (c + 1) * OL)
        w_c = W[:, :, sl].transpose(1, 0, 2).reshape(F, SH).astype(bf16)
        wg = np.ascontiguousarray(
            w_c.reshape(NG, G, 128, SH).transpose(0, 2, 1, 3).reshape(NG, 128, WB)
        )
        xwg = np.concatenate([xg, wg], axis=2)
        # attn broadcast tiles: [128, 768] = (l | r), (h, o) col order
        attn_c = np.concatenate(
            [attn_l[:, sl].reshape(-1), attn_r[:, sl].reshape(-1)]
        ).astype(bf16)
        attn_bc = np.ascontiguousarray(
            np.broadcast_to(attn_c.reshape(1, 2 * SH), (128, 2 * SH))
        )
        fcw_c = np.zeros((NP, 2 * SH), dtype=np.float32)
        fcw_c[:N, :] = fcv[:, :, :, sl].transpose(1, 0, 2, 3).reshape(N, 2 * SH)
        fcwp = np.ascontiguousarray(
            fcw_c.reshape(4, 128, 2 * SH).transpose(1, 0, 2).reshape(128, 8 * SH)
        ).astype(bf16)
        in_maps.append(
            {"xw": xwg, "aftp": aftp, "afp": afp, "attn": attn_bc,
             "fcwp": fcwp, "fcb": fcb}
        )
    return in_maps


def _ensure_ntff_hook():
    """The agent image's antenv lacks axon_hooks; register the profile hook
    ourselves so run_bass_kernel_spmd(trace=True) can collect NTFF profiles."""
    import types

    try:
        from antenv.axon_hooks import get_axon_ntff_profile_hook  # noqa: F401
        return
    except ImportError:
        pass
    try:
        import antenv
        from trn_agent_boot.trn_boot import _ntff_profile_via_ctypes

        mod = types.ModuleType("antenv.axon_hooks")
        _hook = [_ntff_profile_via_ctypes("/opt/axon/libaxon_pjrt.so")]
        mod.set_axon_ntff_profile_hook = lambda h: _hook.__setitem__(0, h)
        mod.get_axon_ntff_profile_hook = lambda: _hook[0]
        sys.modules["antenv.axon_hooks"] = mod
        antenv.axon_hooks = mod
    except Exception:
        pass


def kernel(A, x, W, attn_l, attn_r, fc_w, fc_b):
    global LAST_EXEC_NS, LAST_RESULT
    from concourse.bass_utils import run_bass_kernel_spmd

    if os.environ.get("BASS_TRACE"):
        _ensure_ntff_hook()

    in_maps = prepare_in_maps(A, x, W, attn_l, attn_r, fc_w, fc_b)
    nc = build(stage=int(os.environ.get("BASS_STAGE", "99")))
    res = run_bass_kernel_spmd(
        nc, in_maps, core_ids=list(range(NC)),
        trace=bool(os.environ.get("BASS_TRACE")),
    )
    LAST_EXEC_NS = res.exec_time_ns
    LAST_RESULT = res
    out = res.results[0]["out"]
    return np.asarray(out).reshape(-1)[:2].astype(np.float32)
